# revision 1
# baseline (speedup 1.0000x reference)
"""DigitCaps dynamic-routing kernel for 8 TRN2 NeuronCores.

x (1024, 1152, 8) f32, W (1152, 8, 10, 16) f32 -> v (1024, 10, 16) f32,
3 routing iterations. Pure data-parallel over batch (128 samples/core),
W replicated. The 10 output classes are independent, so the kernel runs
class-major: per class all three routing iterations complete using small
per-class state (beta, exp(beta)).

Layouts (SBUF partition starts must be 0/32/64/96, so 16-row structures
are handled in 32-row class/tile pairs):
  - (r,i)-tiles of 128 on partitions for PE contractions
  - W^T stored as class-pairs (32 rows) at 32-aligned partition bases;
    the h-matmul uses K=32 with a zero-padded v operand
  - beta/exp(beta) stored in 32-row tile-pair blocks; selector/expansion
    matmuls use parity-split 32-row selector matrices
  - b=128 on partitions for softmax-normalize/squash
"""

import numpy as np

B, R, I, C, O = 1024, 1152, 8, 10, 16
RI = R * I            # 9216
CO = C * O            # 160
NT = RI // 128        # 72 (r,i)-tiles
NCORES = 8
BC = B // NCORES      # 128
N_ITERS = 3

_COMPILED = {}


def _build_kernel(bf16_logits=False, nt=NT):
    import contextlib

    import concourse.bass as bass
    import concourse.bacc as bacc
    import concourse.tile as tile
    from concourse import mybir
    from concourse.masks import make_identity

    f32 = mybir.dt.float32
    lgt = mybir.dt.bfloat16 if bf16_logits else f32
    AF = mybir.ActivationFunctionType
    nc = bacc.Bacc()
    NT_ = nt
    NG_ = max(1, nt // 8)   # beta col groups; tile-pair pi -> base 32*(pi%4), col pi//4
    RI_ = nt * 128
    R_ = nt * 16

    xt_d = nc.declare_dram_parameter("xt", [128, NT_, 128], f32, isOutput=False)
    wm_d = nc.declare_dram_parameter("wm", [128, NT_, CO], f32, isOutput=False)
    wtp_d = nc.declare_dram_parameter("wtp", [128, 2 * RI_], lgt, isOutput=False)
    sel_d = nc.declare_dram_parameter("sel2", [128, 2, 32], lgt, isOutput=False)
    ex_d = nc.declare_dram_parameter("ex4", [128, 2, 128], lgt, isOutput=False)
    out_d = nc.declare_dram_parameter("out", [128, CO], f32, isOutput=True)

    with tile.TileContext(nc) as tc:
        with contextlib.ExitStack() as ctx:
            singles = ctx.enter_context(tc.tile_pool(name="singles", bufs=1))
            small = ctx.enter_context(tc.tile_pool(name="small", bufs=2))
            work = ctx.enter_context(tc.tile_pool(name="work", bufs=4))
            ps_ph = ctx.enter_context(tc.tile_pool(name="ps_ph", bufs=2, space="PSUM"))
            ps_py = ctx.enter_context(tc.tile_pool(name="ps_py", bufs=2, space="PSUM"))
            ps_pb = ctx.enter_context(tc.tile_pool(name="ps_pb", bufs=2, space="PSUM"))
            ps_mi = ctx.enter_context(tc.tile_pool(name="ps_mi", bufs=1, space="PSUM"))

            xt = singles.tile([128, NT_, 128], f32)    # [p=(r,i), t, b]
            wm = singles.tile([128, NT_, CO], f32)     # [p=(r,i), t, (c,o)]
            wtp = singles.tile([128, 2 * RI_], lgt)    # W^T class-pairs
            sel2 = singles.tile([128, 2, 32], lgt)     # parity selectors
            ex4 = singles.tile([128, 2, 128], lgt)     # parity expanders x4 bases
            ones = singles.tile([128, 1], lgt)
            iden = singles.tile([128, 128], f32)
            bt = singles.tile([128, NG_, 128], f32)    # beta (one class at a time)
            et = singles.tile([128, NG_, 128], lgt)    # exp(beta)
            v_ext = singles.tile([128, 4, 32], f32)    # zero-padded v, 4 copies
            vtc = singles.tile([128, 128], lgt)        # v^T pair operand x4 bases
            s0_sb = singles.tile([128, CO], f32)       # s0 all classes, b on parts
            v_all = singles.tile([128, CO], f32)       # final v, b on parts

            nc.sync.dma_start(out=xt, in_=xt_d[:])
            nc.sync.dma_start(out=wm, in_=wm_d[:])
            nc.sync.dma_start(out=wtp, in_=wtp_d[:])
            nc.sync.dma_start(out=sel2, in_=sel_d[:])
            nc.sync.dma_start(out=ex4, in_=ex_d[:])
            nc.vector.memset(ones, 1.0)
            make_identity(nc, iden)

            # Absorber matmuls: each waits on exactly one input DMA so no
            # later matmul joins >1 semaphore (walrus allows 1 wait/LDW).
            for src_ap in (
                xt[:, 0, 0:1], wm[:, 0, 0:1], wtp[:, 0:1],
                sel2[:, 0, 0:1], ex4[:, 0, 0:1], iden[:, 0:1],
            ):
                jp = ps_pb.tile([1, 1], f32, tag="pb")
                nc.tensor.matmul(jp, src_ap, src_ap, start=True, stop=True)

            def wt_slice(c, t):
                q = c // 2
                base = 32 * (q % 4)
                col = (q // 4) * RI_
                return wtp[base : base + 32, col + 128 * t : col + 128 * t + 128]

            def bc(ap2, n):
                """broadcast a [128, 1] AP over a new innermost dim of size n"""
                return bass.AP(
                    tensor=ap2.tensor, offset=ap2.offset,
                    ap=[list(ap2.ap[0]), [0, n]],
                )

            def bc4(ap2, inner):
                """[128, X] AP -> [128, (0,4), X-dims] broadcast over copy dim"""
                return bass.AP(
                    tensor=ap2.tensor, offset=ap2.offset,
                    ap=[list(ap2.ap[0]), [0, 4]]
                    + ([list(d) for d in ap2.ap[1:]] if not inner else [[0, O]]),
                )

            def _squash_core(sc_ap):
                """returns fac [128,1] tile for squash(sc_ap)"""
                sq = small.tile([128, O], f32, tag="sq")
                nc.vector.tensor_mul(sq, sc_ap, sc_ap)
                nrm = small.tile([128, 1], f32, tag="nrm")
                nc.vector.tensor_reduce(
                    nrm, sq, axis=mybir.AxisListType.X, op=mybir.AluOpType.add
                )
                rt = small.tile([128, 1], f32, tag="rt")
                nc.scalar.sqrt(rt, nrm)
                np1 = small.tile([128, 1], f32, tag="np1")
                nc.scalar.add(np1, nrm, 1.0)
                den = small.tile([128, 1], f32, tag="den")
                nc.vector.tensor_mul(den, np1, rt)
                rf = small.tile([128, 1], f32, tag="rf")
                nc.vector.reciprocal(rf, den)
                fac = small.tile([128, 1], f32, tag="fac")
                nc.vector.tensor_mul(fac, nrm, rf)
                return fac

            def squash_c(sc_ap, v_dst):
                fac = _squash_core(sc_ap)
                nc.vector.tensor_mul(v_dst, sc_ap, bc(fac, O))

            def squash_c4(sc_ap, v_dst4):
                fac = _squash_core(sc_ap)
                nc.vector.tensor_mul(v_dst4, bc4(sc_ap, False), bc4(fac, True))

            def v_to_vtc(c):
                """zero other half of v_ext copies, transpose to vtc x4."""
                half = c % 2
                nc.vector.memset(
                    v_ext[:, :, 16 * (1 - half) : 16 * (1 - half) + 16], 0.0
                )
                pvt = ps_mi.tile([128, 128], f32, tag="tp")
                nc.tensor.transpose(
                    pvt, v_ext.rearrange("p a b -> p (a b)"), iden
                )
                nc.scalar.copy(vtc, pvt)

            # s0 for all classes: one K=9216 accumulation chain
            ps0 = ps_mi.tile([128, CO], f32, tag="acc")
            for t in range(NT_):
                nc.tensor.matmul(
                    ps0, xt[:, t, :], wm[:, t, :],
                    start=(t == 0), stop=(t == NT_ - 1),
                )
            nc.scalar.activation(s0_sb, ps0, AF.Copy, scale=1.0 / R_)

            for c in range(C):
                half = c % 2
                vslice = v_ext[:, :, 16 * half : 16 * half + 16]
                # ---- iter 0 ----
                squash_c4(s0_sb[:, 16 * c : 16 * c + 16], vslice)
                v_to_vtc(c)

                for it in (1, 2):
                    # ---- beta update: tiles in pairs ----
                    for pi in range(NT_ // 2):
                        pb32 = ps_pb.tile([32, 128], f32, tag="pb")
                        for par in (0, 1):
                            t = 2 * pi + par
                            ph = ps_ph.tile([128, 128], f32, tag="ph")
                            qb = 32 * ((c // 2) % 4)
                            nc.tensor.matmul(
                                ph, wt_slice(c, t), vtc[qb : qb + 32, :],
                                start=True, stop=True,
                                tile_position=(qb, 0),
                            )
                            xh = work.tile([128, 128], lgt, tag="xh")
                            nc.vector.tensor_mul(xh, ph, xt[:, t, :])
                            nc.tensor.matmul(
                                pb32, sel2[:, par, :], xh,
                                start=(par == 0), stop=(par == 1),
                            )
                        base = 32 * (pi % 4)
                        dst = bt[base : base + 32, pi // 4, :]
                        if it == 1:
                            nc.scalar.copy(dst, pb32)
                        else:
                            nc.vector.tensor_add(dst, dst, pb32)
                    # ---- exp + denominator ----
                    nc.scalar.activation(
                        et.rearrange("p g b -> p (g b)"),
                        bt.rearrange("p g b -> p (g b)"),
                        AF.Exp,
                    )
                    pd = ps_mi.tile([1, 128], f32, tag="tp")
                    for g in range(NG_):
                        nc.tensor.matmul(
                            pd, ones, et[:, g, :],
                            start=(g == 0), stop=(g == NG_ - 1),
                        )
                    # ---- s numerator ----
                    psc = ps_mi.tile([16, 128], f32, tag="acc")
                    for t in range(NT_):
                        pi, par = t // 2, t % 2
                        py = ps_py.tile([128, 128], f32, tag="py")
                        eb = 32 * (pi % 4)
                        nc.tensor.matmul(
                            py, ex4[eb : eb + 32, par, :],
                            et[eb : eb + 32, pi // 4, :],
                            start=True, stop=True,
                            tile_position=(eb, 0),
                        )
                        y = work.tile([128, 128], f32, tag="y")
                        nc.vector.tensor_mul(y, py, xt[:, t, :])
                        nc.tensor.matmul(
                            psc, wm[:, t, 16 * c : 16 * c + 16], y,
                            start=(t == 0), stop=(t == NT_ - 1),
                        )
                    # ---- transpose s_num and denom to b-partitions ----
                    scT = small.tile([16, 128], f32, tag="scT")
                    nc.scalar.copy(scT, psc)
                    dcol = small.tile([1, 128], f32, tag="dcol")
                    nc.scalar.copy(dcol, pd)
                    pss = ps_mi.tile([128, 16], f32, tag="acc")
                    nc.tensor.transpose(pss, scT, iden[0:16, 0:16])
                    psd = ps_mi.tile([128, 1], f32, tag="tp")
                    nc.tensor.transpose(psd, dcol, iden[0:1, 0:1])
                    dinv = small.tile([128, 1], f32, tag="dinv")
                    nc.vector.reciprocal(dinv, psd)
                    sc_n = small.tile([128, O], f32, tag="sc_n")
                    nc.vector.tensor_mul(sc_n, pss, bc(dinv, O))
                    # ---- squash ----
                    if it < N_ITERS - 1:
                        squash_c4(sc_n, vslice)
                        v_to_vtc(c)
                    else:
                        squash_c(sc_n, v_all[:, 16 * c : 16 * c + 16])

            nc.sync.dma_start(out=out_d[:], in_=v_all)

    nc.finalize()
    return nc


def make_consts(lg):
    p = np.arange(128)
    j = np.arange(32)
    sel2 = np.zeros((128, 2, 32), dtype=np.float32)
    ex2 = np.zeros((32, 2, 128), dtype=np.float32)
    for par in range(2):
        sel2[:, par, :] = (j[None, :] // 16 == par) & (
            p[:, None] // 8 == j[None, :] % 16
        )
        ex2[:, par, :] = (j[:, None] // 16 == par) & (
            j[:, None] % 16 == p[None, :] // 8
        )
    ex4 = np.tile(ex2, (4, 1, 1))  # replicate at bases 0/32/64/96
    return sel2.astype(lg), ex4.astype(lg)


def pack_wtp(W_mat, nt, lg):
    """W^T (CO, RI_) -> [128, 2*RI_] class-pair layout."""
    RI_ = nt * 128
    WT = np.ascontiguousarray(W_mat.T)
    wtp = np.zeros((128, 2 * RI_), dtype=np.float32)
    for q in range(5):
        base = 32 * (q % 4)
        col = (q // 4) * RI_
        wtp[base : base + 32, col : col + RI_] = WT[32 * q : 32 * q + 32]
    return wtp.astype(lg)


def _prep_inputs(x, W, bf16_logits=False):
    import ml_dtypes

    lg = ml_dtypes.bfloat16 if bf16_logits else np.float32
    W_mat = np.ascontiguousarray(W.reshape(RI, CO), dtype=np.float32)
    wm_h = np.ascontiguousarray(W_mat.reshape(NT, 128, CO).transpose(1, 0, 2))
    wtp_h = pack_wtp(W_mat, NT, lg)
    sel2_h, ex4_h = make_consts(lg)

    in_maps = []
    for k in range(NCORES):
        x_flat = x[k * BC : (k + 1) * BC].reshape(BC, RI)
        xt_h = np.ascontiguousarray(
            x_flat.T.reshape(NT, 128, BC).transpose(1, 0, 2), dtype=np.float32
        )
        in_maps.append(
            {"xt": xt_h, "wm": wm_h, "wtp": wtp_h, "sel2": sel2_h, "ex4": ex4_h}
        )
    return in_maps


def kernel(x, W, _trace=False, _bf16=False):
    from concourse.bass_utils import run_bass_kernel_spmd

    key = f"k{_bf16}"
    if key not in _COMPILED:
        _COMPILED[key] = _build_kernel(bf16_logits=_bf16)
    nc = _COMPILED[key]

    in_maps = _prep_inputs(np.asarray(x), np.asarray(W), bf16_logits=_bf16)
    res = run_bass_kernel_spmd(nc, in_maps, list(range(NCORES)), trace=_trace)
    outs = [res.results[k]["out"] for k in range(NCORES)]
    v = np.concatenate(outs, axis=0).reshape(B, C, O).astype(np.float32)
    if _trace:
        return v, res
    return v



# revision 7
# speedup vs baseline: 6.0928x; 6.0928x over previous
"""DigitCaps dynamic-routing kernel for 8 TRN2 NeuronCores.

x (1024, 1152, 8) f32, W (1152, 8, 10, 16) f32 -> v (1024, 10, 16) f32,
3 routing iterations. Pure data-parallel over batch (128 samples/core).
The 10 output classes are independent, so the kernel runs class-major:
per class all three routing iterations complete using small per-class
state (beta, exp(beta)).

Host->device transfer over the axon tunnel (~40 MB/s) dominates
end-to-end time, so the wire format is minimized:
  - x and W ship as fp16 (rel err ~3e-3 through the routing loop,
    gate is 2e-2); all routing-state math stays f32 on device.
  - W ships SHARDED: core k gets 9 of the 72 (r,i)-tiles; an on-device
    AllGather reconstructs the full W on every core (5.9 MB f32 -> 0.37
    MB fp16 per core on the wire).
  - W^T (wtp, the class-pair layout for the beta-update matmul) is
    derived on device via PE transposes instead of being shipped.

Layouts (SBUF partition starts must be 0/32/64/96, so 16-row structures
are handled in 32-row class/tile pairs):
  - (r,i)-tiles of 128 on partitions for PE contractions
  - W^T stored as class-pairs (32 rows) at 32-aligned partition bases;
    the h-matmul uses K=32 with a zero-padded v operand
  - beta/exp(beta) stored in 32-row tile-pair blocks; selector/expansion
    matmuls use parity-split 32-row selector matrices
  - b=128 on partitions for softmax-normalize/squash
"""

import numpy as np

B, R, I, C, O = 1024, 1152, 8, 10, 16
RI = R * I            # 9216
CO = C * O            # 160
NT = RI // 128        # 72 (r,i)-tiles
NCORES = 8
BC = B // NCORES      # 128
N_ITERS = 3
NTS = NT // NCORES    # 9 W-tiles per core on the wire
SHARD_W = True

_COMPILED = {}


def _build_kernel(bf16_logits=False, nt=NT, shard_w=None):
    import contextlib

    import concourse.bass as bass
    import concourse.bacc as bacc
    import concourse.tile as tile
    from concourse import mybir
    from concourse.masks import make_identity

    if shard_w is None:
        shard_w = SHARD_W
    f32 = mybir.dt.float32
    f16 = mybir.dt.float16
    AF = mybir.ActivationFunctionType
    nc = bacc.Bacc(num_devices=NCORES)
    NT_ = nt
    NG_ = max(1, nt // 8)   # beta col groups; tile-pair pi -> base 32*(pi%4), col pi//4
    RI_ = nt * 128
    R_ = nt * 16
    NTS_ = NT_ // NCORES

    xt_d = nc.declare_dram_parameter("xt", [128, NT_, 128], f16, isOutput=False)
    if shard_w:
        wm_d = nc.declare_dram_parameter("wm", [128, NTS_, CO], f16, isOutput=False)
        wsh_b = nc.dram_tensor("wsh_b", [128, NTS_, CO], f16)
        wg_b = nc.dram_tensor(
            "wg_b", [NCORES, 128, NTS_, CO], f16, addr_space="Shared"
        )
    else:
        wm_d = nc.declare_dram_parameter("wm", [128, NT_, CO], f16, isOutput=False)
    sel_d = nc.declare_dram_parameter("sel2", [128, 2, 32], f16, isOutput=False)
    ex_d = nc.declare_dram_parameter("ex4", [128, 2, 128], f16, isOutput=False)
    out_d = nc.declare_dram_parameter("out", [128, CO], f32, isOutput=True)

    with tile.TileContext(nc) as tc:
        with contextlib.ExitStack() as ctx:
            singles = ctx.enter_context(tc.tile_pool(name="singles", bufs=1))
            small = ctx.enter_context(tc.tile_pool(name="small", bufs=2))
            work = ctx.enter_context(tc.tile_pool(name="work", bufs=4))
            ps_ph = ctx.enter_context(tc.tile_pool(name="ps_ph", bufs=2, space="PSUM"))
            ps_py = ctx.enter_context(tc.tile_pool(name="ps_py", bufs=2, space="PSUM"))
            ps_pb = ctx.enter_context(tc.tile_pool(name="ps_pb", bufs=2, space="PSUM"))
            ps_mi = ctx.enter_context(tc.tile_pool(name="ps_mi", bufs=1, space="PSUM"))

            xt = singles.tile([128, NT_, 128], f16)    # [p=(r,i), t, b]
            wm16 = singles.tile([128, NT_, CO], f16)   # [p=(r,i), t, (c,o)]
            wm32 = singles.tile([128, NT_, CO], f32)   # f32 copy for s-numerator
            wtp = singles.tile([128, 2 * RI_], f16)    # W^T class-pairs (derived)
            sel2 = singles.tile([128, 2, 32], f16)     # parity selectors
            ex4h = singles.tile([128, 2, 128], f16)    # parity expanders (wire)
            ex4 = singles.tile([128, 2, 128], f32)     # f32 copy for py matmul
            ones = singles.tile([128, 1], f32)
            iden = singles.tile([128, 128], f32)
            bt = singles.tile([128, NG_, 128], f32)    # beta (one class at a time)
            et = singles.tile([128, NG_, 128], f32)    # exp(beta)
            v_ext = singles.tile([128, 4, 32], f32)    # zero-padded v, 4 copies
            vtc = singles.tile([128, 128], f16)        # v^T pair operand x4 bases
            s0_sb = singles.tile([128, CO], f32)       # s0 all classes, b on parts
            v_all = singles.tile([128, CO], f32)       # final v, b on parts

            nc.sync.dma_start(out=xt, in_=xt_d[:])
            if shard_w:
                nc.sync.dma_start(out=wsh_b[:, :, :], in_=wm_d[:])
                nc.gpsimd.collective_compute(
                    "AllGather",
                    mybir.AluOpType.bypass,
                    replica_groups=[list(range(NCORES))],
                    ins=[wsh_b.ap().opt()],
                    outs=[wg_b.ap().opt()],
                )
                for a in range(NCORES):
                    nc.sync.dma_start(
                        out=wm16[:, a * NTS_ : (a + 1) * NTS_, :],
                        in_=wg_b[a, :, :, :],
                    )
            else:
                nc.sync.dma_start(out=wm16, in_=wm_d[:])
            nc.sync.dma_start(out=sel2, in_=sel_d[:])
            nc.sync.dma_start(out=ex4h, in_=ex_d[:])
            nc.vector.memset(ones, 1.0)
            make_identity(nc, iden)

            # Absorber matmuls: each waits on exactly one input DMA so no
            # later matmul joins >1 semaphore (walrus allows 1 wait/LDW).
            for src_ap in (
                xt[:, 0, 0:1], wm16[:, 0, 0:1], sel2[:, 0, 0:1],
                ex4h[:, 0, 0:1], iden[:, 0:1],
            ):
                jp = ps_pb.tile([1, 1], f32, tag="pb")
                nc.tensor.matmul(jp, src_ap, src_ap, start=True, stop=True)

            # fp16 wire -> f32 working copies (scalar engine, one-time)
            nc.scalar.copy(
                wm32.rearrange("p t c -> p (t c)"),
                wm16.rearrange("p t c -> p (t c)"),
            )
            nc.scalar.copy(
                ex4.rearrange("p a b -> p (a b)"),
                ex4h.rearrange("p a b -> p (a b)"),
            )

            def wt_slice(c, t):
                q = c // 2
                base = 32 * (q % 4)
                col = (q // 4) * RI_
                return wtp[base : base + 32, col + 128 * t : col + 128 * t + 128]

            def bc(ap2, n):
                """broadcast a [128, 1] AP over a new innermost dim of size n"""
                return bass.AP(
                    tensor=ap2.tensor, offset=ap2.offset,
                    ap=[list(ap2.ap[0]), [0, n]],
                )

            def bc4(ap2, inner):
                """[128, X] AP -> [128, (0,4), X-dims] broadcast over copy dim"""
                return bass.AP(
                    tensor=ap2.tensor, offset=ap2.offset,
                    ap=[list(ap2.ap[0]), [0, 4]]
                    + ([list(d) for d in ap2.ap[1:]] if not inner else [[0, O]]),
                )

            def _squash_core(sc_ap):
                """returns fac [128,1] tile for squash(sc_ap)"""
                sq = small.tile([128, O], f32, tag="sq")
                nc.vector.tensor_mul(sq, sc_ap, sc_ap)
                nrm = small.tile([128, 1], f32, tag="nrm")
                nc.vector.tensor_reduce(
                    nrm, sq, axis=mybir.AxisListType.X, op=mybir.AluOpType.add
                )
                rt = small.tile([128, 1], f32, tag="rt")
                nc.scalar.sqrt(rt, nrm)
                np1 = small.tile([128, 1], f32, tag="np1")
                nc.scalar.add(np1, nrm, 1.0)
                den = small.tile([128, 1], f32, tag="den")
                nc.vector.tensor_mul(den, np1, rt)
                rf = small.tile([128, 1], f32, tag="rf")
                nc.vector.reciprocal(rf, den)
                fac = small.tile([128, 1], f32, tag="fac")
                nc.vector.tensor_mul(fac, nrm, rf)
                return fac

            def squash_c(sc_ap, v_dst):
                fac = _squash_core(sc_ap)
                nc.vector.tensor_mul(v_dst, sc_ap, bc(fac, O))

            def squash_c4(sc_ap, v_dst4):
                fac = _squash_core(sc_ap)
                nc.vector.tensor_mul(v_dst4, bc4(sc_ap, False), bc4(fac, True))

            def v_to_vtc(c):
                """zero other half of v_ext copies, transpose to vtc x4."""
                half = c % 2
                nc.vector.memset(
                    v_ext[:, :, 16 * (1 - half) : 16 * (1 - half) + 16], 0.0
                )
                pvt = ps_mi.tile([128, 128], f32, tag="tp")
                nc.tensor.transpose(
                    pvt, v_ext.rearrange("p a b -> p (a b)"), iden
                )
                nc.scalar.copy(vtc, pvt)

            # s0 for all classes: one K=9216 accumulation chain
            ps0 = ps_mi.tile([128, CO], f32, tag="acc")
            for t in range(NT_):
                nc.tensor.matmul(
                    ps0, xt[:, t, :], wm16[:, t, :],
                    start=(t == 0), stop=(t == NT_ - 1),
                )
            nc.scalar.activation(s0_sb, ps0, AF.Copy, scale=1.0 / R_)

            # derive wtp (W^T class-pair layout) from wm32 via PE transposes
            # (f32 in/out, reusing the ph/pb PSUM tags; the scalar copy
            # downconverts to the fp16 wtp operand, exact since wm is fp16):
            # classes 0..7 land on partitions 0..127 of the first RI_ cols,
            # classes 8..9 on partitions 0..31 of the second RI_ cols.
            for t in range(NT_):
                ptA = ps_ph.tile([128, 128], f32, tag="ph")
                nc.tensor.transpose(ptA, wm32[:, t, 0:128], iden)
                nc.scalar.copy(wtp[:, 128 * t : 128 * t + 128], ptA)
                ptB = ps_pb.tile([32, 128], f32, tag="pb")
                nc.tensor.transpose(ptB, wm32[:, t, 128:160], iden)
                nc.scalar.copy(
                    wtp[0:32, RI_ + 128 * t : RI_ + 128 * t + 128], ptB
                )

            for c in range(C):
                half = c % 2
                vslice = v_ext[:, :, 16 * half : 16 * half + 16]
                # ---- iter 0 ----
                squash_c4(s0_sb[:, 16 * c : 16 * c + 16], vslice)
                v_to_vtc(c)

                for it in (1, 2):
                    # ---- beta update: tiles in pairs ----
                    for pi in range(NT_ // 2):
                        pb32 = ps_pb.tile([32, 128], f32, tag="pb")
                        for par in (0, 1):
                            t = 2 * pi + par
                            ph = ps_ph.tile([128, 128], f32, tag="ph")
                            qb = 32 * ((c // 2) % 4)
                            nc.tensor.matmul(
                                ph, wt_slice(c, t), vtc[qb : qb + 32, :],
                                start=True, stop=True,
                                tile_position=(qb, 0),
                            )
                            xh = work.tile([128, 128], f16, tag="xh")
                            nc.vector.tensor_mul(xh, ph, xt[:, t, :])
                            nc.tensor.matmul(
                                pb32, sel2[:, par, :], xh,
                                start=(par == 0), stop=(par == 1),
                            )
                        base = 32 * (pi % 4)
                        dst = bt[base : base + 32, pi // 4, :]
                        if it == 1:
                            nc.scalar.copy(dst, pb32)
                        else:
                            nc.vector.tensor_add(dst, dst, pb32)
                    # ---- exp + denominator ----
                    nc.scalar.activation(
                        et.rearrange("p g b -> p (g b)"),
                        bt.rearrange("p g b -> p (g b)"),
                        AF.Exp,
                    )
                    pd = ps_mi.tile([1, 128], f32, tag="tp")
                    for g in range(NG_):
                        nc.tensor.matmul(
                            pd, ones, et[:, g, :],
                            start=(g == 0), stop=(g == NG_ - 1),
                        )
                    # ---- s numerator ----
                    psc = ps_mi.tile([16, 128], f32, tag="acc")
                    for t in range(NT_):
                        pi, par = t // 2, t % 2
                        py = ps_py.tile([128, 128], f32, tag="py")
                        eb = 32 * (pi % 4)
                        nc.tensor.matmul(
                            py, ex4[eb : eb + 32, par, :],
                            et[eb : eb + 32, pi // 4, :],
                            start=True, stop=True,
                            tile_position=(eb, 0),
                        )
                        y = work.tile([128, 128], f32, tag="y")
                        nc.vector.tensor_mul(y, py, xt[:, t, :])
                        nc.tensor.matmul(
                            psc, wm32[:, t, 16 * c : 16 * c + 16], y,
                            start=(t == 0), stop=(t == NT_ - 1),
                        )
                    # ---- transpose s_num and denom to b-partitions ----
                    scT = small.tile([16, 128], f32, tag="scT")
                    nc.scalar.copy(scT, psc)
                    dcol = small.tile([1, 128], f32, tag="dcol")
                    nc.scalar.copy(dcol, pd)
                    pss = ps_mi.tile([128, 16], f32, tag="acc")
                    nc.tensor.transpose(pss, scT, iden[0:16, 0:16])
                    psd = ps_mi.tile([128, 1], f32, tag="tp")
                    nc.tensor.transpose(psd, dcol, iden[0:1, 0:1])
                    dinv = small.tile([128, 1], f32, tag="dinv")
                    nc.vector.reciprocal(dinv, psd)
                    sc_n = small.tile([128, O], f32, tag="sc_n")
                    nc.vector.tensor_mul(sc_n, pss, bc(dinv, O))
                    # ---- squash ----
                    if it < N_ITERS - 1:
                        squash_c4(sc_n, vslice)
                        v_to_vtc(c)
                    else:
                        squash_c(sc_n, v_all[:, 16 * c : 16 * c + 16])

            nc.sync.dma_start(out=out_d[:], in_=v_all)

    nc.finalize()
    return nc


def make_consts(lg):
    p = np.arange(128)
    j = np.arange(32)
    sel2 = np.zeros((128, 2, 32), dtype=np.float32)
    ex2 = np.zeros((32, 2, 128), dtype=np.float32)
    for par in range(2):
        sel2[:, par, :] = (j[None, :] // 16 == par) & (
            p[:, None] // 8 == j[None, :] % 16
        )
        ex2[:, par, :] = (j[:, None] // 16 == par) & (
            j[:, None] % 16 == p[None, :] // 8
        )
    ex4 = np.tile(ex2, (4, 1, 1))  # replicate at bases 0/32/64/96
    return sel2.astype(lg), ex4.astype(lg)


def _prep_inputs(x, W, bf16_logits=False):
    lg = np.float16
    W_mat = np.ascontiguousarray(W.reshape(RI, CO), dtype=np.float32)
    wm_h = np.ascontiguousarray(
        W_mat.reshape(NT, 128, CO).transpose(1, 0, 2), dtype=lg
    )
    sel2_h, ex4_h = make_consts(lg)

    in_maps = []
    for k in range(NCORES):
        x_flat = x[k * BC : (k + 1) * BC].reshape(BC, RI)
        xt_h = np.ascontiguousarray(
            x_flat.T.reshape(NT, 128, BC).transpose(1, 0, 2), dtype=lg
        )
        wm_k = (
            np.ascontiguousarray(wm_h[:, k * NTS : (k + 1) * NTS, :])
            if SHARD_W
            else wm_h
        )
        in_maps.append(
            {"xt": xt_h, "wm": wm_k, "sel2": sel2_h, "ex4": ex4_h}
        )
    return in_maps


def kernel(x, W, _trace=False, _bf16=False):
    from concourse.bass_utils import run_bass_kernel_spmd

    key = "k"
    if key not in _COMPILED:
        _COMPILED[key] = _build_kernel()
    nc = _COMPILED[key]

    in_maps = _prep_inputs(np.asarray(x), np.asarray(W))
    res = run_bass_kernel_spmd(nc, in_maps, list(range(NCORES)), trace=_trace)
    outs = [res.results[k]["out"] for k in range(NCORES)]
    v = np.concatenate(outs, axis=0).reshape(B, C, O).astype(np.float32)
    if _trace:
        return v, res
    return v


# revision 9
# speedup vs baseline: 7.2768x; 1.1943x over previous
"""DigitCaps dynamic-routing kernel for 8 TRN2 NeuronCores.

x (1024, 1152, 8) f32, W (1152, 8, 10, 16) f32 -> v (1024, 10, 16) f32,
3 routing iterations. Pure data-parallel over batch (128 samples/core).
The 10 output classes are independent, so the kernel runs class-major:
per class all three routing iterations complete using small per-class
state (beta, exp(beta)).

Host->device transfer over the axon tunnel (~40 MB/s) dominates
end-to-end time, so the wire format is minimized (rel err ~5e-3 through
the routing loop, gate is 2e-2):
  - x ships as int12 fixed-point (scale = absmax/2047, folded into W
    host-side): a full int8 "hi" plane (v>>4) plus packed lo nibbles
    (samples b and b+64 share a byte). On-device unpack: xt_f16 =
    16*hi + lo.
  - W ships fp16 and SHARDED: core k gets 9 of the 72 (r,i)-tiles; an
    on-device AllGather reconstructs the full W on every core.
  - W^T (wtp, class-pair layout for the beta-update matmul) is derived
    on device via PE transposes; the sel2/ex4 selector constants are
    generated on device (affine_select + PE transpose).
All routing-state math stays f32.

Layouts (SBUF partition starts must be 0/32/64/96, so 16-row structures
are handled in 32-row class/tile pairs):
  - (r,i)-tiles of 128 on partitions for PE contractions
  - W^T stored as class-pairs (32 rows) at 32-aligned partition bases;
    the h-matmul uses K=32 with a zero-padded v operand
  - beta/exp(beta) stored in 32-row tile-pair blocks; selector/expansion
    matmuls use parity-split 32-row selector matrices
  - b=128 on partitions for softmax-normalize/squash
"""

import numpy as np

B, R, I, C, O = 1024, 1152, 8, 10, 16
RI = R * I            # 9216
CO = C * O            # 160
NT = RI // 128        # 72 (r,i)-tiles
NCORES = 8
BC = B // NCORES      # 128
N_ITERS = 3
NTS = NT // NCORES    # 9 W-tiles per core on the wire
TCH = 8               # unpack chunk (tiles)
SHARD_W = True

_COMPILED = {}


def _build_kernel(bf16_logits=False, nt=NT, shard_w=None):
    import contextlib

    import concourse.bass as bass
    import concourse.bacc as bacc
    import concourse.tile as tile
    from concourse import mybir
    from concourse.masks import make_identity

    if shard_w is None:
        shard_w = SHARD_W
    f32 = mybir.dt.float32
    f16 = mybir.dt.float16
    i8 = mybir.dt.int8
    u8 = mybir.dt.uint8
    AF = mybir.ActivationFunctionType
    ALU = mybir.AluOpType
    nc = bacc.Bacc(num_devices=NCORES)
    NT_ = nt
    NG_ = max(1, nt // 8)   # beta col groups; tile-pair pi -> base 32*(pi%4), col pi//4
    RI_ = nt * 128
    R_ = nt * 16
    NTS_ = NT_ // NCORES

    xh_d = nc.declare_dram_parameter("xh", [128, NT_, 128], i8, isOutput=False)
    xl_d = nc.declare_dram_parameter("xl", [128, NT_, 64], u8, isOutput=False)
    if shard_w:
        wm_d = nc.declare_dram_parameter("wm", [128, NTS_, CO], f16, isOutput=False)
        wsh_b = nc.dram_tensor("wsh_b", [128, NTS_, CO], f16)
        wg_b = nc.dram_tensor(
            "wg_b", [NCORES, 128, NTS_, CO], f16, addr_space="Shared"
        )
    else:
        wm_d = nc.declare_dram_parameter("wm", [128, NT_, CO], f16, isOutput=False)
    out_d = nc.declare_dram_parameter("out", [128, CO], f32, isOutput=True)

    with tile.TileContext(nc) as tc:
        with contextlib.ExitStack() as ctx:
            singles = ctx.enter_context(tc.tile_pool(name="singles", bufs=1))
            small = ctx.enter_context(tc.tile_pool(name="small", bufs=2))
            work = ctx.enter_context(tc.tile_pool(name="work", bufs=4))
            ps_ph = ctx.enter_context(tc.tile_pool(name="ps_ph", bufs=2, space="PSUM"))
            ps_py = ctx.enter_context(tc.tile_pool(name="ps_py", bufs=2, space="PSUM"))
            ps_pb = ctx.enter_context(tc.tile_pool(name="ps_pb", bufs=2, space="PSUM"))
            ps_mi = ctx.enter_context(tc.tile_pool(name="ps_mi", bufs=1, space="PSUM"))

            xh = singles.tile([128, NT_, 128], i8)     # x int12 high 8 bits
            xl = singles.tile([128, NT_, 64], u8)      # x lo nibbles (b, b+64)
            xt = singles.tile([128, NT_, 128], f16)    # [p=(r,i), t, b] unpacked
            wm16 = singles.tile([128, NT_, CO], f16)   # [p=(r,i), t, (c,o)]
            wm32 = singles.tile([128, NT_, CO], f32)   # f32 copy for s-numerator
            wtp = singles.tile([128, 2 * RI_], f16)    # W^T class-pairs (derived)
            sel2f = singles.tile([128, 2, 2, 16], f32) # selector (generated)
            sel2 = singles.tile([128, 2, 32], f16)     # fp16 copy for beta matmul
            ex4 = singles.tile([128, 2, 128], f32)     # parity expanders (derived)
            ones = singles.tile([128, 1], f32)
            iden = singles.tile([128, 128], f32)
            bt = singles.tile([128, NG_, 128], f32)    # beta (one class at a time)
            et = singles.tile([128, NG_, 128], f32)    # exp(beta)
            v_ext = singles.tile([128, 4, 32], f32)    # zero-padded v, 4 copies
            vtc = singles.tile([128, 128], f16)        # v^T pair operand x4 bases
            s0_sb = singles.tile([128, CO], f32)       # s0 all classes, b on parts
            v_all = singles.tile([128, CO], f32)       # final v, b on parts

            nc.sync.dma_start(out=xh, in_=xh_d[:])
            nc.sync.dma_start(out=xl, in_=xl_d[:])
            if shard_w:
                nc.sync.dma_start(out=wsh_b[:, :, :], in_=wm_d[:])
                nc.gpsimd.collective_compute(
                    "AllGather",
                    mybir.AluOpType.bypass,
                    replica_groups=[list(range(NCORES))],
                    ins=[wsh_b.ap().opt()],
                    outs=[wg_b.ap().opt()],
                )
                for a in range(NCORES):
                    nc.sync.dma_start(
                        out=wm16[:, a * NTS_ : (a + 1) * NTS_, :],
                        in_=wg_b[a, :, :, :],
                    )
            else:
                nc.sync.dma_start(out=wm16, in_=wm_d[:])
            nc.vector.memset(ones, 1.0)
            make_identity(nc, iden)

            # sel2[p, par, j] = (j//16 == par) & (p//8 == j%16), j = 16*jhi+jlo
            nc.gpsimd.memset(sel2f, 1.0)
            nc.gpsimd.affine_select(
                out=sel2f, in_=sel2f, compare_op=ALU.is_ge, fill=0.0,
                base=0, channel_multiplier=1,
                pattern=[[0, 2], [0, 2], [-8, 16]],   # p - 8*jlo >= 0
            )
            nc.gpsimd.affine_select(
                out=sel2f, in_=sel2f, compare_op=ALU.is_ge, fill=0.0,
                base=7, channel_multiplier=-1,
                pattern=[[0, 2], [0, 2], [8, 16]],    # 7 - p + 8*jlo >= 0
            )
            nc.gpsimd.affine_select(
                out=sel2f, in_=sel2f, compare_op=ALU.is_ge, fill=0.0,
                base=0, channel_multiplier=0,
                pattern=[[-1, 2], [1, 2], [0, 16]],   # jhi - par >= 0
            )
            nc.gpsimd.affine_select(
                out=sel2f, in_=sel2f, compare_op=ALU.is_ge, fill=0.0,
                base=0, channel_multiplier=0,
                pattern=[[1, 2], [-1, 2], [0, 16]],   # par - jhi >= 0
            )

            # Absorber matmuls: each waits on exactly one input's writer so
            # no later matmul joins >1 semaphore (walrus allows 1 wait/LDW).
            for src_ap in (wm16[:, NT_ - 1, 0:1], iden[:, 0:1]):
                jp = ps_pb.tile([1, 1], f32, tag="pb")
                nc.tensor.matmul(jp, src_ap, src_ap, start=True, stop=True)

            # fp16/selector wire -> f32 working copies (scalar engine, one-time)
            nc.scalar.copy(
                wm32.rearrange("p t c -> p (t c)"),
                wm16.rearrange("p t c -> p (t c)"),
            )
            nc.scalar.copy(
                sel2.rearrange("p a j -> p (a j)"),
                sel2f.rearrange("p a h l -> p (a h l)"),
            )

            # unpack x: xt = 16*hi + lo (DVE is the sole writer of xt)
            for t0 in range(0, NT_, TCH):
                tc_sl = slice(t0, t0 + TCH)
                hi_f = work.tile([128, TCH, 128], f16, tag="hi_f")
                nc.scalar.activation(
                    hi_f.rearrange("p t b -> p (t b)"),
                    xh[:, tc_sl, :].rearrange("p t b -> p (t b)"),
                    AF.Copy, scale=16.0,
                )
                lo1u = work.tile([128, TCH, 64], u8, tag="lo1u")
                nc.vector.tensor_scalar(
                    lo1u, xl[:, tc_sl, :], 4, None, ALU.logical_shift_right
                )
                lo0u = work.tile([128, TCH, 64], u8, tag="lo0u")
                nc.vector.tensor_scalar(
                    lo0u, xl[:, tc_sl, :], 15, None, ALU.bitwise_and
                )
                lo1f = work.tile([128, TCH, 64], f16, tag="lo1f")
                nc.scalar.copy(lo1f, lo1u)
                lo0f = work.tile([128, TCH, 64], f16, tag="lo0f")
                nc.scalar.copy(lo0f, lo0u)
                nc.vector.tensor_add(
                    xt[:, tc_sl, 0:64], hi_f[:, :, 0:64], lo0f
                )
                nc.vector.tensor_add(
                    xt[:, tc_sl, 64:128], hi_f[:, :, 64:128], lo1f
                )

            def wt_slice(c, t):
                q = c // 2
                base = 32 * (q % 4)
                col = (q // 4) * RI_
                return wtp[base : base + 32, col + 128 * t : col + 128 * t + 128]

            def bc(ap2, n):
                """broadcast a [128, 1] AP over a new innermost dim of size n"""
                return bass.AP(
                    tensor=ap2.tensor, offset=ap2.offset,
                    ap=[list(ap2.ap[0]), [0, n]],
                )

            def bc4(ap2, inner):
                """[128, X] AP -> [128, (0,4), X-dims] broadcast over copy dim"""
                return bass.AP(
                    tensor=ap2.tensor, offset=ap2.offset,
                    ap=[list(ap2.ap[0]), [0, 4]]
                    + ([list(d) for d in ap2.ap[1:]] if not inner else [[0, O]]),
                )

            def _squash_core(sc_ap):
                """returns fac [128,1] tile for squash(sc_ap)"""
                sq = small.tile([128, O], f32, tag="sq")
                nc.vector.tensor_mul(sq, sc_ap, sc_ap)
                nrm = small.tile([128, 1], f32, tag="nrm")
                nc.vector.tensor_reduce(
                    nrm, sq, axis=mybir.AxisListType.X, op=mybir.AluOpType.add
                )
                rt = small.tile([128, 1], f32, tag="rt")
                nc.scalar.sqrt(rt, nrm)
                np1 = small.tile([128, 1], f32, tag="np1")
                nc.scalar.add(np1, nrm, 1.0)
                den = small.tile([128, 1], f32, tag="den")
                nc.vector.tensor_mul(den, np1, rt)
                rf = small.tile([128, 1], f32, tag="rf")
                nc.vector.reciprocal(rf, den)
                fac = small.tile([128, 1], f32, tag="fac")
                nc.vector.tensor_mul(fac, nrm, rf)
                return fac

            def squash_c(sc_ap, v_dst):
                fac = _squash_core(sc_ap)
                nc.vector.tensor_mul(v_dst, sc_ap, bc(fac, O))

            def squash_c4(sc_ap, v_dst4):
                fac = _squash_core(sc_ap)
                nc.vector.tensor_mul(v_dst4, bc4(sc_ap, False), bc4(fac, True))

            def v_to_vtc(c):
                """zero other half of v_ext copies, transpose to vtc x4."""
                half = c % 2
                nc.vector.memset(
                    v_ext[:, :, 16 * (1 - half) : 16 * (1 - half) + 16], 0.0
                )
                pvt = ps_mi.tile([128, 128], f32, tag="tp")
                nc.tensor.transpose(
                    pvt, v_ext.rearrange("p a b -> p (a b)"), iden
                )
                nc.scalar.copy(vtc, pvt)

            # derive wtp (W^T class-pair layout) from wm32 via PE transposes
            # (f32 in/out, reusing the ph/pb PSUM tags; the scalar copy
            # downconverts to the fp16 wtp operand, exact since wm is fp16):
            # classes 0..7 land on partitions 0..127 of the first RI_ cols,
            # classes 8..9 on partitions 0..31 of the second RI_ cols.
            for t in range(NT_):
                ptA = ps_ph.tile([128, 128], f32, tag="ph")
                nc.tensor.transpose(ptA, wm32[:, t, 0:128], iden)
                nc.scalar.copy(wtp[:, 128 * t : 128 * t + 128], ptA)
                ptB = ps_pb.tile([32, 128], f32, tag="pb")
                nc.tensor.transpose(ptB, wm32[:, t, 128:160], iden)
                nc.scalar.copy(
                    wtp[0:32, RI_ + 128 * t : RI_ + 128 * t + 128], ptB
                )

            # derive ex4 = sel2^T replicated at partition bases 0/32/64/96
            for par in range(2):
                pse = ps_pb.tile([32, 128], f32, tag="pb")
                nc.tensor.transpose(
                    pse, sel2f.rearrange("p a h l -> p a (h l)")[:, par, :], iden
                )
                for rep in range(4):
                    nc.scalar.copy(ex4[32 * rep : 32 * rep + 32, par, :], pse)

            # absorber for xt (sole writer: DVE) before the s0 chain
            jp = ps_pb.tile([1, 1], f32, tag="pb")
            nc.tensor.matmul(
                jp, xt[:, NT_ - 1, 127:128], xt[:, NT_ - 1, 127:128],
                start=True, stop=True,
            )

            # s0 for all classes: one K=9216 accumulation chain
            ps0 = ps_mi.tile([128, CO], f32, tag="acc")
            for t in range(NT_):
                nc.tensor.matmul(
                    ps0, xt[:, t, :], wm16[:, t, :],
                    start=(t == 0), stop=(t == NT_ - 1),
                )
            nc.scalar.activation(s0_sb, ps0, AF.Copy, scale=1.0 / R_)

            for c in range(C):
                half = c % 2
                vslice = v_ext[:, :, 16 * half : 16 * half + 16]
                # ---- iter 0 ----
                squash_c4(s0_sb[:, 16 * c : 16 * c + 16], vslice)
                v_to_vtc(c)

                for it in (1, 2):
                    # ---- beta update: tiles in pairs ----
                    for pi in range(NT_ // 2):
                        pb32 = ps_pb.tile([32, 128], f32, tag="pb")
                        for par in (0, 1):
                            t = 2 * pi + par
                            ph = ps_ph.tile([128, 128], f32, tag="ph")
                            qb = 32 * ((c // 2) % 4)
                            nc.tensor.matmul(
                                ph, wt_slice(c, t), vtc[qb : qb + 32, :],
                                start=True, stop=True,
                                tile_position=(qb, 0),
                            )
                            xh_w = work.tile([128, 128], f16, tag="xh")
                            nc.vector.tensor_mul(xh_w, ph, xt[:, t, :])
                            nc.tensor.matmul(
                                pb32, sel2[:, par, :], xh_w,
                                start=(par == 0), stop=(par == 1),
                            )
                        base = 32 * (pi % 4)
                        dst = bt[base : base + 32, pi // 4, :]
                        if it == 1:
                            nc.scalar.copy(dst, pb32)
                        else:
                            nc.vector.tensor_add(dst, dst, pb32)
                    # ---- exp + denominator ----
                    nc.scalar.activation(
                        et.rearrange("p g b -> p (g b)"),
                        bt.rearrange("p g b -> p (g b)"),
                        AF.Exp,
                    )
                    pd = ps_mi.tile([1, 128], f32, tag="tp")
                    for g in range(NG_):
                        nc.tensor.matmul(
                            pd, ones, et[:, g, :],
                            start=(g == 0), stop=(g == NG_ - 1),
                        )
                    # ---- s numerator ----
                    psc = ps_mi.tile([16, 128], f32, tag="acc")
                    for t in range(NT_):
                        pi, par = t // 2, t % 2
                        py = ps_py.tile([128, 128], f32, tag="py")
                        eb = 32 * (pi % 4)
                        nc.tensor.matmul(
                            py, ex4[eb : eb + 32, par, :],
                            et[eb : eb + 32, pi // 4, :],
                            start=True, stop=True,
                            tile_position=(eb, 0),
                        )
                        y = work.tile([128, 128], f32, tag="y")
                        nc.vector.tensor_mul(y, py, xt[:, t, :])
                        nc.tensor.matmul(
                            psc, wm32[:, t, 16 * c : 16 * c + 16], y,
                            start=(t == 0), stop=(t == NT_ - 1),
                        )
                    # ---- transpose s_num and denom to b-partitions ----
                    scT = small.tile([16, 128], f32, tag="scT")
                    nc.scalar.copy(scT, psc)
                    dcol = small.tile([1, 128], f32, tag="dcol")
                    nc.scalar.copy(dcol, pd)
                    pss = ps_mi.tile([128, 16], f32, tag="acc")
                    nc.tensor.transpose(pss, scT, iden[0:16, 0:16])
                    psd = ps_mi.tile([128, 1], f32, tag="tp")
                    nc.tensor.transpose(psd, dcol, iden[0:1, 0:1])
                    dinv = small.tile([128, 1], f32, tag="dinv")
                    nc.vector.reciprocal(dinv, psd)
                    sc_n = small.tile([128, O], f32, tag="sc_n")
                    nc.vector.tensor_mul(sc_n, pss, bc(dinv, O))
                    # ---- squash ----
                    if it < N_ITERS - 1:
                        squash_c4(sc_n, vslice)
                        v_to_vtc(c)
                    else:
                        squash_c(sc_n, v_all[:, 16 * c : 16 * c + 16])

            nc.sync.dma_start(out=out_d[:], in_=v_all)

    nc.finalize()
    return nc


def _prep_inputs(x, W, bf16_logits=False):
    x = np.asarray(x, dtype=np.float32)
    s12 = float(np.abs(x).max()) / 2047.0
    W_mat = np.ascontiguousarray(W.reshape(RI, CO), dtype=np.float32) * s12
    wm_h = np.ascontiguousarray(
        W_mat.reshape(NT, 128, CO).transpose(1, 0, 2), dtype=np.float16
    )

    in_maps = []
    for k in range(NCORES):
        x_flat = x[k * BC : (k + 1) * BC].reshape(BC, RI)
        xt_f = np.ascontiguousarray(
            x_flat.T.reshape(NT, 128, BC).transpose(1, 0, 2), dtype=np.float32
        )
        v = np.clip(np.round(xt_f / s12), -2047, 2047).astype(np.int16)
        xh_h = (v >> 4).astype(np.int8)
        lo = (v & 15).astype(np.uint8)
        xl_h = (lo[:, :, 0:64] | (lo[:, :, 64:128] << 4)).astype(np.uint8)
        wm_k = (
            np.ascontiguousarray(wm_h[:, k * NTS : (k + 1) * NTS, :])
            if SHARD_W
            else wm_h
        )
        in_maps.append({"xh": xh_h, "xl": xl_h, "wm": wm_k})
    return in_maps


def kernel(x, W, _trace=False, _bf16=False):
    from concourse.bass_utils import run_bass_kernel_spmd

    key = "k"
    if key not in _COMPILED:
        _COMPILED[key] = _build_kernel()
    nc = _COMPILED[key]

    in_maps = _prep_inputs(np.asarray(x), np.asarray(W))
    res = run_bass_kernel_spmd(nc, in_maps, list(range(NCORES)), trace=_trace)
    outs = [res.results[k]["out"] for k in range(NCORES)]
    v = np.concatenate(outs, axis=0).reshape(B, C, O).astype(np.float32)
    if _trace:
        return v, res
    return v


# revision 12
# speedup vs baseline: 7.4631x; 1.0256x over previous
"""DigitCaps dynamic-routing kernel for 8 TRN2 NeuronCores.

x (1024, 1152, 8) f32, W (1152, 8, 10, 16) f32 -> v (1024, 10, 16) f32,
3 routing iterations. Pure data-parallel over batch (128 samples/core).
The 10 output classes are independent, so the kernel runs class-major:
per class all three routing iterations complete using small per-class
state (beta, exp(beta)).

Host->device transfer over the axon tunnel (~40 MB/s) dominates
end-to-end time, so the wire format is minimized (rel err ~5e-3 through
the routing loop, gate is 2e-2):
  - x ships as int12 fixed-point (scale = absmax/2047, folded into W
    host-side): a full int8 "hi" plane (v>>4) plus packed lo nibbles
    (samples b and b+64 share a byte). On-device unpack: xt_f16 =
    16*hi + lo.
  - W ships fp16 and SHARDED: core k gets 9 of the 72 (r,i)-tiles; an
    on-device AllGather reconstructs the full W on every core.
  - W^T (wtp, class-pair layout for the beta-update matmul) is derived
    on device via PE transposes; the sel2/ex4 selector constants are
    generated on device (affine_select + PE transpose).
All routing-state math stays f32.

Layouts (SBUF partition starts must be 0/32/64/96, so 16-row structures
are handled in 32-row class/tile pairs):
  - (r,i)-tiles of 128 on partitions for PE contractions
  - W^T stored as class-pairs (32 rows) at 32-aligned partition bases;
    the h-matmul uses K=32 with a zero-padded v operand
  - beta/exp(beta) stored in 32-row tile-pair blocks; selector/expansion
    matmuls use parity-split 32-row selector matrices
  - b=128 on partitions for softmax-normalize/squash
"""

import numpy as np

B, R, I, C, O = 1024, 1152, 8, 10, 16
RI = R * I            # 9216
CO = C * O            # 160
NT = RI // 128        # 72 (r,i)-tiles
NCORES = 8
BC = B // NCORES      # 128
N_ITERS = 3
NTS = NT // NCORES    # 9 W-tiles per core on the wire
TCH = 8               # unpack chunk (tiles)
SHARD_W = True

_COMPILED = {}


def _build_kernel(bf16_logits=False, nt=NT, shard_w=None):
    import contextlib

    import concourse.bass as bass
    import concourse.bacc as bacc
    import concourse.tile as tile
    from concourse import mybir
    from concourse.masks import make_identity

    if shard_w is None:
        shard_w = SHARD_W
    f32 = mybir.dt.float32
    f16 = mybir.dt.float16
    i8 = mybir.dt.int8
    u8 = mybir.dt.uint8
    AF = mybir.ActivationFunctionType
    ALU = mybir.AluOpType
    nc = bacc.Bacc(num_devices=NCORES)
    NT_ = nt
    NG_ = max(1, nt // 8)   # beta col groups; tile-pair pi -> base 32*(pi%4), col pi//4
    RI_ = nt * 128
    R_ = nt * 16
    NTS_ = NT_ // NCORES

    # single uint8 wire blob per core: xh | xl | wm-shard bytes
    XHB = NT_ * 128
    XLB = NT_ * 64
    WMB = (NTS_ if shard_w else NT_) * CO * 2
    blob_d = nc.declare_dram_parameter(
        "blob", [128, XHB + XLB + WMB], u8, isOutput=False
    )
    if shard_w:
        wsh_b = nc.dram_tensor("wsh_b", [128, NTS_ * CO], f16)
        wg_b = nc.dram_tensor(
            "wg_b", [NCORES, 128, NTS_ * CO], f16, addr_space="Shared"
        )
    out_d = nc.declare_dram_parameter("out", [128, CO], f16, isOutput=True)

    with tile.TileContext(nc) as tc:
        with contextlib.ExitStack() as ctx:
            singles = ctx.enter_context(tc.tile_pool(name="singles", bufs=1))
            small = ctx.enter_context(tc.tile_pool(name="small", bufs=2))
            work = ctx.enter_context(tc.tile_pool(name="work", bufs=4))
            ps_ph = ctx.enter_context(tc.tile_pool(name="ps_ph", bufs=2, space="PSUM"))
            ps_py = ctx.enter_context(tc.tile_pool(name="ps_py", bufs=2, space="PSUM"))
            ps_pb = ctx.enter_context(tc.tile_pool(name="ps_pb", bufs=2, space="PSUM"))
            ps_mi = ctx.enter_context(tc.tile_pool(name="ps_mi", bufs=1, space="PSUM"))

            xh = singles.tile([128, NT_, 128], i8)     # x int12 high 8 bits
            xl = singles.tile([128, NT_, 64], u8)      # x lo nibbles (b, b+64)
            xt = singles.tile([128, NT_, 128], f16)    # [p=(r,i), t, b] unpacked
            wm16 = singles.tile([128, NT_, CO], f16)   # [p=(r,i), t, (c,o)]
            wm32 = singles.tile([128, NT_, CO], f32)   # f32 copy for s-numerator
            wtp = singles.tile([128, 2 * RI_], f16)    # W^T class-pairs (derived)
            sel2f = singles.tile([128, 2, 2, 16], f32) # selector (generated)
            sel2 = singles.tile([128, 2, 32], f16)     # fp16 copy for beta matmul
            ex4 = singles.tile([128, 2, 128], f32)     # parity expanders (derived)
            ones = singles.tile([128, 1], f32)
            iden = singles.tile([128, 128], f32)
            bt = singles.tile([128, NG_, 128], f32)    # beta (one class at a time)
            et = singles.tile([128, NG_, 128], f32)    # exp(beta)
            v_ext = singles.tile([128, 4, 32], f32)    # zero-padded v, 4 copies
            vtc = singles.tile([128, 128], f16)        # v^T pair operand x4 bases
            s0_sb = singles.tile([128, CO], f32)       # s0 all classes, b on parts
            v_all = singles.tile([128, CO], f16)       # final v, b on parts

            wm16_f = wm16.rearrange("p t c -> p (t c)")
            nc.sync.dma_start(
                out=xh.rearrange("p t b -> p (t b)"),
                in_=blob_d[:, 0:XHB].bitcast(i8),
            )
            nc.sync.dma_start(
                out=xl.rearrange("p t b -> p (t b)"),
                in_=blob_d[:, XHB : XHB + XLB],
            )
            wm_in = blob_d[:, XHB + XLB : XHB + XLB + WMB].bitcast(f16)
            if shard_w:
                nc.sync.dma_start(out=wsh_b[:, :], in_=wm_in)
                nc.gpsimd.collective_compute(
                    "AllGather",
                    mybir.AluOpType.bypass,
                    replica_groups=[list(range(NCORES))],
                    ins=[wsh_b.ap().opt()],
                    outs=[wg_b.ap().opt()],
                )
                for a in range(NCORES):
                    nc.sync.dma_start(
                        out=wm16_f[:, a * NTS_ * CO : (a + 1) * NTS_ * CO],
                        in_=wg_b[a, :, :],
                    )
            else:
                nc.sync.dma_start(out=wm16_f, in_=wm_in)
            nc.vector.memset(ones, 1.0)
            make_identity(nc, iden)

            # sel2[p, par, j] = (j//16 == par) & (p//8 == j%16), j = 16*jhi+jlo
            nc.gpsimd.memset(sel2f, 1.0)
            nc.gpsimd.affine_select(
                out=sel2f, in_=sel2f, compare_op=ALU.is_ge, fill=0.0,
                base=0, channel_multiplier=1,
                pattern=[[0, 2], [0, 2], [-8, 16]],   # p - 8*jlo >= 0
            )
            nc.gpsimd.affine_select(
                out=sel2f, in_=sel2f, compare_op=ALU.is_ge, fill=0.0,
                base=7, channel_multiplier=-1,
                pattern=[[0, 2], [0, 2], [8, 16]],    # 7 - p + 8*jlo >= 0
            )
            nc.gpsimd.affine_select(
                out=sel2f, in_=sel2f, compare_op=ALU.is_ge, fill=0.0,
                base=0, channel_multiplier=0,
                pattern=[[-1, 2], [1, 2], [0, 16]],   # jhi - par >= 0
            )
            nc.gpsimd.affine_select(
                out=sel2f, in_=sel2f, compare_op=ALU.is_ge, fill=0.0,
                base=0, channel_multiplier=0,
                pattern=[[1, 2], [-1, 2], [0, 16]],   # par - jhi >= 0
            )

            # Absorber matmuls: each waits on exactly one input's writer so
            # no later matmul joins >1 semaphore (walrus allows 1 wait/LDW).
            for src_ap in (wm16[:, NT_ - 1, 0:1], iden[:, 0:1]):
                jp = ps_pb.tile([1, 1], f32, tag="pb")
                nc.tensor.matmul(jp, src_ap, src_ap, start=True, stop=True)

            # fp16/selector wire -> f32 working copies (scalar engine, one-time)
            nc.scalar.copy(
                wm32.rearrange("p t c -> p (t c)"),
                wm16.rearrange("p t c -> p (t c)"),
            )
            nc.scalar.copy(
                sel2.rearrange("p a j -> p (a j)"),
                sel2f.rearrange("p a h l -> p (a h l)"),
            )

            # unpack x: xt = 16*hi + lo (DVE is the sole writer of xt)
            for t0 in range(0, NT_, TCH):
                tc_sl = slice(t0, t0 + TCH)
                hi_f = work.tile([128, TCH, 128], f16, tag="hi_f")
                nc.scalar.activation(
                    hi_f.rearrange("p t b -> p (t b)"),
                    xh[:, tc_sl, :].rearrange("p t b -> p (t b)"),
                    AF.Copy, scale=16.0,
                )
                lo1u = work.tile([128, TCH, 64], u8, tag="lo1u")
                nc.vector.tensor_scalar(
                    lo1u, xl[:, tc_sl, :], 4, None, ALU.logical_shift_right
                )
                lo0u = work.tile([128, TCH, 64], u8, tag="lo0u")
                nc.vector.tensor_scalar(
                    lo0u, xl[:, tc_sl, :], 15, None, ALU.bitwise_and
                )
                lo1f = work.tile([128, TCH, 64], f16, tag="lo1f")
                nc.scalar.copy(lo1f, lo1u)
                lo0f = work.tile([128, TCH, 64], f16, tag="lo0f")
                nc.scalar.copy(lo0f, lo0u)
                nc.vector.tensor_add(
                    xt[:, tc_sl, 0:64], hi_f[:, :, 0:64], lo0f
                )
                nc.vector.tensor_add(
                    xt[:, tc_sl, 64:128], hi_f[:, :, 64:128], lo1f
                )

            def wt_slice(c, t):
                q = c // 2
                base = 32 * (q % 4)
                col = (q // 4) * RI_
                return wtp[base : base + 32, col + 128 * t : col + 128 * t + 128]

            def bc(ap2, n):
                """broadcast a [128, 1] AP over a new innermost dim of size n"""
                return bass.AP(
                    tensor=ap2.tensor, offset=ap2.offset,
                    ap=[list(ap2.ap[0]), [0, n]],
                )

            def bc4(ap2, inner):
                """[128, X] AP -> [128, (0,4), X-dims] broadcast over copy dim"""
                return bass.AP(
                    tensor=ap2.tensor, offset=ap2.offset,
                    ap=[list(ap2.ap[0]), [0, 4]]
                    + ([list(d) for d in ap2.ap[1:]] if not inner else [[0, O]]),
                )

            def _squash_core(sc_ap):
                """returns fac [128,1] tile for squash(sc_ap)"""
                sq = small.tile([128, O], f32, tag="sq")
                nc.vector.tensor_mul(sq, sc_ap, sc_ap)
                nrm = small.tile([128, 1], f32, tag="nrm")
                nc.vector.tensor_reduce(
                    nrm, sq, axis=mybir.AxisListType.X, op=mybir.AluOpType.add
                )
                rt = small.tile([128, 1], f32, tag="rt")
                nc.scalar.sqrt(rt, nrm)
                np1 = small.tile([128, 1], f32, tag="np1")
                nc.scalar.add(np1, nrm, 1.0)
                den = small.tile([128, 1], f32, tag="den")
                nc.vector.tensor_mul(den, np1, rt)
                rf = small.tile([128, 1], f32, tag="rf")
                nc.vector.reciprocal(rf, den)
                fac = small.tile([128, 1], f32, tag="fac")
                nc.vector.tensor_mul(fac, nrm, rf)
                return fac

            def squash_c(sc_ap, v_dst):
                fac = _squash_core(sc_ap)
                nc.vector.tensor_mul(v_dst, sc_ap, bc(fac, O))

            def squash_c4(sc_ap, v_dst4):
                fac = _squash_core(sc_ap)
                nc.vector.tensor_mul(v_dst4, bc4(sc_ap, False), bc4(fac, True))

            def v_to_vtc(c):
                """zero other half of v_ext copies, transpose to vtc x4."""
                half = c % 2
                nc.vector.memset(
                    v_ext[:, :, 16 * (1 - half) : 16 * (1 - half) + 16], 0.0
                )
                pvt = ps_mi.tile([128, 128], f32, tag="tp")
                nc.tensor.transpose(
                    pvt, v_ext.rearrange("p a b -> p (a b)"), iden
                )
                nc.scalar.copy(vtc, pvt)

            # derive wtp (W^T class-pair layout) from wm32 via PE transposes
            # (f32 in/out, reusing the ph/pb PSUM tags; the scalar copy
            # downconverts to the fp16 wtp operand, exact since wm is fp16):
            # classes 0..7 land on partitions 0..127 of the first RI_ cols,
            # classes 8..9 on partitions 0..31 of the second RI_ cols.
            for t in range(NT_):
                ptA = ps_ph.tile([128, 128], f32, tag="ph")
                nc.tensor.transpose(ptA, wm32[:, t, 0:128], iden)
                nc.scalar.copy(wtp[:, 128 * t : 128 * t + 128], ptA)
                ptB = ps_pb.tile([32, 128], f32, tag="pb")
                nc.tensor.transpose(ptB, wm32[:, t, 128:160], iden)
                nc.scalar.copy(
                    wtp[0:32, RI_ + 128 * t : RI_ + 128 * t + 128], ptB
                )

            # derive ex4 = sel2^T replicated at partition bases 0/32/64/96
            for par in range(2):
                pse = ps_pb.tile([32, 128], f32, tag="pb")
                nc.tensor.transpose(
                    pse, sel2f.rearrange("p a h l -> p a (h l)")[:, par, :], iden
                )
                for rep in range(4):
                    nc.scalar.copy(ex4[32 * rep : 32 * rep + 32, par, :], pse)

            # absorber for xt (sole writer: DVE) before the s0 chain
            jp = ps_pb.tile([1, 1], f32, tag="pb")
            nc.tensor.matmul(
                jp, xt[:, NT_ - 1, 127:128], xt[:, NT_ - 1, 127:128],
                start=True, stop=True,
            )

            # s0 for all classes: one K=9216 accumulation chain
            ps0 = ps_mi.tile([128, CO], f32, tag="acc")
            for t in range(NT_):
                nc.tensor.matmul(
                    ps0, xt[:, t, :], wm16[:, t, :],
                    start=(t == 0), stop=(t == NT_ - 1),
                )
            nc.scalar.activation(s0_sb, ps0, AF.Copy, scale=1.0 / R_)

            for c in range(C):
                half = c % 2
                vslice = v_ext[:, :, 16 * half : 16 * half + 16]
                # ---- iter 0 ----
                squash_c4(s0_sb[:, 16 * c : 16 * c + 16], vslice)
                v_to_vtc(c)

                for it in (1, 2):
                    # ---- beta update: tiles in pairs ----
                    for pi in range(NT_ // 2):
                        pb32 = ps_pb.tile([32, 128], f32, tag="pb")
                        for par in (0, 1):
                            t = 2 * pi + par
                            ph = ps_ph.tile([128, 128], f32, tag="ph")
                            qb = 32 * ((c // 2) % 4)
                            nc.tensor.matmul(
                                ph, wt_slice(c, t), vtc[qb : qb + 32, :],
                                start=True, stop=True,
                                tile_position=(qb, 0),
                            )
                            xh_w = work.tile([128, 128], f16, tag="xh")
                            nc.vector.tensor_mul(xh_w, ph, xt[:, t, :])
                            nc.tensor.matmul(
                                pb32, sel2[:, par, :], xh_w,
                                start=(par == 0), stop=(par == 1),
                            )
                        base = 32 * (pi % 4)
                        dst = bt[base : base + 32, pi // 4, :]
                        if it == 1:
                            nc.scalar.copy(dst, pb32)
                        else:
                            nc.vector.tensor_add(dst, dst, pb32)
                    # ---- exp + denominator ----
                    nc.scalar.activation(
                        et.rearrange("p g b -> p (g b)"),
                        bt.rearrange("p g b -> p (g b)"),
                        AF.Exp,
                    )
                    pd = ps_mi.tile([1, 128], f32, tag="tp")
                    for g in range(NG_):
                        nc.tensor.matmul(
                            pd, ones, et[:, g, :],
                            start=(g == 0), stop=(g == NG_ - 1),
                        )
                    # ---- s numerator ----
                    psc = ps_mi.tile([16, 128], f32, tag="acc")
                    for t in range(NT_):
                        pi, par = t // 2, t % 2
                        py = ps_py.tile([128, 128], f32, tag="py")
                        eb = 32 * (pi % 4)
                        nc.tensor.matmul(
                            py, ex4[eb : eb + 32, par, :],
                            et[eb : eb + 32, pi // 4, :],
                            start=True, stop=True,
                            tile_position=(eb, 0),
                        )
                        y = work.tile([128, 128], f32, tag="y")
                        nc.vector.tensor_mul(y, py, xt[:, t, :])
                        nc.tensor.matmul(
                            psc, wm32[:, t, 16 * c : 16 * c + 16], y,
                            start=(t == 0), stop=(t == NT_ - 1),
                        )
                    # ---- transpose s_num and denom to b-partitions ----
                    scT = small.tile([16, 128], f32, tag="scT")
                    nc.scalar.copy(scT, psc)
                    dcol = small.tile([1, 128], f32, tag="dcol")
                    nc.scalar.copy(dcol, pd)
                    pss = ps_mi.tile([128, 16], f32, tag="acc")
                    nc.tensor.transpose(pss, scT, iden[0:16, 0:16])
                    psd = ps_mi.tile([128, 1], f32, tag="tp")
                    nc.tensor.transpose(psd, dcol, iden[0:1, 0:1])
                    dinv = small.tile([128, 1], f32, tag="dinv")
                    nc.vector.reciprocal(dinv, psd)
                    sc_n = small.tile([128, O], f32, tag="sc_n")
                    nc.vector.tensor_mul(sc_n, pss, bc(dinv, O))
                    # ---- squash ----
                    if it < N_ITERS - 1:
                        squash_c4(sc_n, vslice)
                        v_to_vtc(c)
                    else:
                        squash_c(sc_n, v_all[:, 16 * c : 16 * c + 16])

            nc.sync.dma_start(out=out_d[:], in_=v_all)

    nc.finalize()
    return nc


def _prep_inputs(x, W, bf16_logits=False):
    x = np.asarray(x, dtype=np.float32)
    s12 = float(np.abs(x).max()) / 2047.0
    W_mat = np.ascontiguousarray(W.reshape(RI, CO), dtype=np.float32) * s12
    wm_h = np.ascontiguousarray(
        W_mat.reshape(NT, 128, CO).transpose(1, 0, 2), dtype=np.float16
    )

    in_maps = []
    for k in range(NCORES):
        x_flat = x[k * BC : (k + 1) * BC].reshape(BC, RI)
        xt_f = np.ascontiguousarray(
            x_flat.T.reshape(NT, 128, BC).transpose(1, 0, 2), dtype=np.float32
        )
        v = np.clip(np.round(xt_f / s12), -2047, 2047).astype(np.int16)
        xh_h = (v >> 4).astype(np.int8)
        lo = (v & 15).astype(np.uint8)
        xl_h = (lo[:, :, 0:64] | (lo[:, :, 64:128] << 4)).astype(np.uint8)
        wm_k = wm_h[:, k * NTS : (k + 1) * NTS, :] if SHARD_W else wm_h
        blob = np.concatenate(
            [
                xh_h.reshape(128, -1).view(np.uint8),
                xl_h.reshape(128, -1),
                np.ascontiguousarray(wm_k).view(np.uint8).reshape(128, -1),
            ],
            axis=1,
        )
        in_maps.append({"blob": blob})
    return in_maps


def kernel(x, W, _trace=False, _bf16=False):
    from concourse.bass_utils import run_bass_kernel_spmd

    key = "k"
    if key not in _COMPILED:
        _COMPILED[key] = _build_kernel()
    nc = _COMPILED[key]

    in_maps = _prep_inputs(np.asarray(x), np.asarray(W))
    res = run_bass_kernel_spmd(nc, in_maps, list(range(NCORES)), trace=_trace)
    outs = [res.results[k]["out"] for k in range(NCORES)]
    v = np.concatenate(outs, axis=0).reshape(B, C, O).astype(np.float32)
    if _trace:
        return v, res
    return v
